# revision 14
# baseline (speedup 1.0000x reference)
"""Causal GQA attention (nkv=1) with RoPE + logit softcap, sharded over 8 trn2 cores.

Sharding: core = 2*b + hh  (b = batch 0..3, hh = head-half 0..1).
Each core computes, for its batch b and its 4 query heads:
  q = rope(x @ Wq_h'.T)          (gain/(sqrt(hd)*softcap) folded into Wq on host)
  k = rope(x @ Wk.T), v = x @ Wv.T   (single kv head, shared across its 4 q heads)
  pT[k,q] = exp(softcap*tanh(qT.k)) * causal_mask   (max-free softmax: softcap
            bounds logits to +-30 so exp never overflows; uniform row scale
            cancels in the normalization)
  outT_h = (v.T @ pT) / sum_k pT    accumulated in PSUM; denominator via ones-matmul
  partial_out[tok, :] = sum_h outT_h.T @ Wo[:, head cols].T
Host sums the two half-head partials per batch and stacks batches.

v4: reverse-skewed chunk schedule. The kernel is ACT-bound in steady state
(tanh+exp over every visible score element ~= 139us); v3 lost ~50us to a
DMA-bound head (first tanh at 27us), ACT idle in the proj-heavy early chunks,
and a 24us tail (chunk-3 Wo + lagged AV after the last exp). v4 processes
q-chunks in order [2, 3, 1, 0]:
  - chunk 2 first: its q needs only x(c2); its tasks touch k-chunks 0..2 at
    ~4us intervals, matching the x DMA stream (c0, c2, c1, c3 priority).
    First tanh ~17us.
  - each chunk's Wo runs as PE filler under the NEXT chunk's ACT stretch
    (wo(2) under chunk 3, wo(3) under chunk 1, wo(1) under chunk 0); only
    wo(0) (the smallest remaining chunk) lands in the tail, staged on the
    then-idle ACT engine, output DMA split across both queues.
  - ACT table set preloaded via a dummy 1x16 tanh+exp during the DMA wait.
  - Q/K rope mul/add on gpsimd (idle ~87% in v3) except the head-critical
    kp(0..2)/qp(2,0) which stay on DVE; PSUM->SBUF casts for kp(0)/qp(2,0)
    ride the pre-first-tanh-idle ACT engine.
  - AV lag 2 mid-kernel, drained to lag 0 through the last (diag-only) chunk.
All matmuls bf16 (1 cyc/row); scores accumulate fp32 in PSUM; tanh keeps
fp32 until the bf16 exp output. Known-failed: fp8/DoubleRow projections
(even one fp8 tensor exceeds the 2e-2 gate), gpsimd pow-as-exp, chunk-3 Wo
head-split, batched per-chunk output DMA.
"""
import numpy as np
import ml_dtypes

import concourse.bacc as bacc
import concourse.mybir as mybir
import concourse.tile as tile
from concourse.bass_utils import run_bass_kernel_spmd

F32 = mybir.dt.float32
BF16 = mybir.dt.bfloat16
NPBF16 = ml_dtypes.bfloat16

B, T, D = 4, 2048, 1024
NH, NKV, HD = 8, 1, 128
SOFTCAP = 30.0
NHL = 4            # heads per core
CH = 512           # q-chunk size
NCH = T // CH      # 4 chunks
NKT = D // 128     # 8 k-tiles over D
NTT = T // 128     # 16 token tiles

# packed column offsets for diagonal groups: k-block j (visible width
# 512-128j) starts at DOFF[j], arranged so every matmul output stays inside
# one 2KB PSUM bank (512 fp32) with zero padding: bank0 = j0(512),
# bank1 = j1(384) + j3(128), bank2 = j2(256). Total 1280 packed columns.
DOFF = [0, 512, 1024, 896]
NWD = 1280         # diag tanh/exp span
DW = 4 * CH        # score tile width (non-diag)

CSEQ = [2, 3, 1, 0]   # chunk processing order (see module docstring)


def _build_nc():
    nc = bacc.Bacc()

    xT = nc.dram_tensor("xT", [D, T], BF16, kind="ExternalInput")
    wqT = nc.dram_tensor("wqT", [D, NHL * HD], BF16, kind="ExternalInput")
    wkT = nc.dram_tensor("wkT", [D, HD], BF16, kind="ExternalInput")
    wvT = nc.dram_tensor("wvT", [D, HD], BF16, kind="ExternalInput")
    woT = nc.dram_tensor("woT", [NHL * HD, D], BF16, kind="ExternalInput")
    cc = nc.dram_tensor("cc", [HD, T], BF16, kind="ExternalInput")
    ssw = nc.dram_tensor("ssw", [HD, T], BF16, kind="ExternalInput")
    tri = nc.dram_tensor("tri", [128, 128], BF16, kind="ExternalInput")
    onesv = nc.dram_tensor("onesv", [128, 128], BF16, kind="ExternalInput")
    out = nc.dram_tensor("out", [T, D], BF16, kind="ExternalOutput")

    xT_t = xT.rearrange("(kt p) t -> p kt t", p=128)      # [128, 8, 2048]
    wqT_t = wqT.rearrange("(kt p) c -> p kt c", p=128)    # [128, 8, 512]
    wkT_t = wkT.rearrange("(kt p) c -> p kt c", p=128)    # [128, 8, 128]
    wvT_t = wvT.rearrange("(kt p) c -> p kt c", p=128)    # [128, 8, 128]
    woT_t = woT.rearrange("(h p) c -> p h c", p=128)      # [128, 4, 1024]

    with tile.TileContext(nc) as tc:
        with (
            tc.tile_pool(name="persist", bufs=1) as persist,
            tc.tile_pool(name="wpool", bufs=1) as wpool,
            tc.tile_pool(name="qt_pool", bufs=2) as qt_pool,
            tc.tile_pool(name="rope_pool", bufs=3) as rope_pool,
            tc.tile_pool(name="p_pool", bufs=5) as p_pool,
            tc.tile_pool(name="pp_pool", bufs=3) as pp_pool,
            tc.tile_pool(name="t4_pool", bufs=1) as t4_pool,
            tc.tile_pool(name="avn_pool", bufs=12) as avn_pool,
            tc.tile_pool(name="osb_pool", bufs=3) as osb_pool,
            tc.tile_pool(name="norm_pool", bufs=2) as norm_pool,
            tc.tile_pool(name="s_pool", bufs=1, space="PSUM") as s_pool,
            tc.tile_pool(name="acc_pool", bufs=1, space="PSUM") as acc_pool,
            tc.tile_pool(name="d_pool", bufs=1, space="PSUM") as d_pool,
            tc.tile_pool(name="pj_pool", bufs=1, space="PSUM") as pj_pool,
            tc.tile_pool(name="pjh_pool", bufs=1, space="PSUM") as pjh_pool,
        ):
            # --- persistent tiles ---
            wq_sb = wpool.tile([128, NKT, NHL * HD], BF16)
            wk_sb = wpool.tile([128, NKT, HD], BF16)
            wv_sb = wpool.tile([128, NKT, HD], BF16)
            wo_sb = wpool.tile([128, NHL, D], BF16)
            cc_sb = wpool.tile([HD, T], BF16)
            ssw_sb = wpool.tile([HD, T], BF16)
            tri_sb = wpool.tile([128, 128], BF16)
            ones_sb = wpool.tile([128, 128], BF16)
            xT_sb = wpool.tile([128, NKT, T], BF16)
            kT_sb = persist.tile([HD, T], BF16)
            v_sb = persist.tile([128, NTT, HD], BF16)

            # ACT table preload: dummy tanh+exp (same table set) so the
            # ~2.7us ACT_TABLE_LOAD runs during the DMA wait, not inside
            # the first real tanh.
            warm = wpool.tile([1, 16], F32)
            nc.vector.memset(warm[:], 0.0)
            warm2 = wpool.tile([1, 16], F32)
            nc.scalar.activation(warm2[:], warm[:],
                                 mybir.ActivationFunctionType.Tanh)
            nc.scalar.activation(warm2[:], warm[:],
                                 mybir.ActivationFunctionType.Exp)

            # --- DMA priorities ---
            # need-times: x(c0)/wk/cc+ssw(c0) ~11us (kp0), x(c2)/wq-h0 ~13
            # (qp(2,0) -> first tanh ~18), x(c1)/wq-h1 ~19, wv ~20,
            # x(c3)/wq-h23 ~28, tri/ones ~22 (first diag exp), wo ~45.
            # The two HWDGE queues stripe over the same 16 engines (~170GB/s
            # each when both have backlog) and issue ~600ns/DMA. x(c0) and
            # x(c2) are interleaved kt-halfwise so the kp(0) and qp(2,0)
            # projection matmuls (in-order PE FIFO) stream as data lands.
            # Scalar-queue issues end by ~13us so the Scalar engine is pure
            # ACT from the first tanh on.
            c0 = slice(0, CH)
            c1 = slice(CH, 2 * CH)
            c2 = slice(2 * CH, 3 * CH)
            c3 = slice(3 * CH, T)
            nc.sync.dma_start(xT_sb[:, 0:1, c0], xT_t[:, 0:1, c0])
            nc.scalar.dma_start(wk_sb[:], wkT_t)
            nc.sync.dma_start(xT_sb[:, 1:4, c0], xT_t[:, 1:4, c0])
            nc.scalar.dma_start(cc_sb[:, c0], cc[:, c0])
            nc.scalar.dma_start(ssw_sb[:, c0], ssw[:, c0])
            nc.sync.dma_start(xT_sb[:, 0:4, c2], xT_t[:, 0:4, c2])
            nc.scalar.dma_start(wq_sb[:, :, 0:HD], wqT_t[:, :, 0:HD])
            nc.sync.dma_start(xT_sb[:, 4:8, c0], xT_t[:, 4:8, c0])
            nc.scalar.dma_start(cc_sb[:, c2], cc[:, c2])
            nc.scalar.dma_start(ssw_sb[:, c2], ssw[:, c2])
            nc.sync.dma_start(xT_sb[:, 4:8, c2], xT_t[:, 4:8, c2])
            nc.scalar.dma_start(wv_sb[:], wvT_t)
            nc.scalar.dma_start(tri_sb[:], tri[:])
            nc.scalar.dma_start(ones_sb[:], onesv[:])
            nc.sync.dma_start(wq_sb[:, :, HD:2 * HD], wqT_t[:, :, HD:2 * HD])
            nc.sync.dma_start(xT_sb[:, :, c1], xT_t[:, :, c1])
            nc.sync.dma_start(cc_sb[:, c1], cc[:, c1])
            nc.sync.dma_start(ssw_sb[:, c1], ssw[:, c1])
            nc.sync.dma_start(wq_sb[:, :, 2 * HD:], wqT_t[:, :, 2 * HD:])
            nc.sync.dma_start(xT_sb[:, :, c3], xT_t[:, :, c3])
            nc.sync.dma_start(cc_sb[:, c3], cc[:, c3])
            nc.sync.dma_start(ssw_sb[:, c3], ssw[:, c3])
            nc.sync.dma_start(wo_sb[:], woT_t)

            def rope_to(dst_ap, src_ps, c, eng, cast_eng):
                """dst = rope(src) for a [128, CH] chunk at token offset c*CH.

                Partition half-swap goes through tensor_copy (TT ops need
                aligned partitions). cast_eng does the PSUM fp32 -> bf16
                read (DVE normally; ACT pre-first-tanh), eng the mul/adds
                (DVE on the latency-critical head path, else gpsimd)."""
                csl = slice(c * CH, (c + 1) * CH)
                qb = rope_pool.tile([128, CH], BF16, tag="qb", name="qb")
                if cast_eng is nc.scalar:
                    nc.scalar.copy(qb[:], src_ps[:])
                else:
                    cast_eng.tensor_copy(qb[:], src_ps[:])
                swp = rope_pool.tile([128, CH], BF16, tag="swp", name="swp")
                nc.vector.tensor_copy(swp[0:64, :], qb[64:128, :])
                nc.vector.tensor_copy(swp[64:128, :], qb[0:64, :])
                m1 = rope_pool.tile([128, CH], BF16, tag="m1", name="m1")
                eng.tensor_mul(m1[:], qb[:], cc_sb[:, csl])
                m2 = rope_pool.tile([128, CH], BF16, tag="m2", name="m2")
                eng.tensor_mul(m2[:], swp[:], ssw_sb[:, csl])
                eng.tensor_add(dst_ap, m1[:], m2[:])

            # rope engine choices: DVE for units on the head critical path
            # (before ~25us, when gpsimd's ~1.2us/op latency would stall
            # the first tasks and DVE is otherwise idle), gpsimd after.
            KP_ENG = {0: (nc.vector, nc.scalar), 1: (nc.vector, None),
                      2: (nc.vector, None), 3: (nc.gpsimd, None)}
            QP_ENG = {(2, 0): (nc.vector, nc.scalar)}

            # ---- filler units (pure-PE work scheduled into ACT-bound gaps) ----
            # Units are split into halves so the paced emitter (below) never
            # inserts more than ~0.9us of PE work between one task's score
            # matmuls and the next's: the PE FIFO is in-order, so a coarse
            # filler in front of the next scores stalls the ACT stream.
            qt_tiles = {}     # c -> qt tile [HD, NHL, CH]
            proj_ps = {}      # ("q",c,h)/("k",c)/("o",c,u) -> live psum tile

            def qp_half(c, h, second):
                csl = slice(c * CH, (c + 1) * CH)
                if not second:
                    if h == 0:
                        qt_tiles[c] = qt_pool.tile([HD, NHL, CH], BF16,
                                                   tag="qt", name="qt")
                    proj_ps[("q", c, h)] = pjh_pool.tile([128, CH], F32,
                                                         tag="pjh", name="q_ps")
                q_ps = proj_ps[("q", c, h)]
                for kt in (range(4) if not second else range(4, NKT)):
                    nc.tensor.matmul(
                        q_ps[0:HD, :], wq_sb[:, kt, h * HD:(h + 1) * HD],
                        xT_sb[:, kt, csl], start=(kt == 0), stop=(kt == NKT - 1))
                if second:
                    eng, cast_eng = QP_ENG.get((c, h), (nc.gpsimd, None))
                    rope_to(qt_tiles[c][:, h, :], q_ps[0:HD, :], c, eng,
                            cast_eng or nc.vector)
                    del proj_ps[("q", c, h)]

            def kp_half(c, second):
                csl = slice(c * CH, (c + 1) * CH)
                if not second:
                    proj_ps[("k", c)] = pjh_pool.tile([128, CH], F32,
                                                      tag="pjh", name="k_ps")
                k_ps = proj_ps[("k", c)]
                for kt in (range(4) if not second else range(4, NKT)):
                    nc.tensor.matmul(k_ps[0:HD, :], wk_sb[:, kt, :],
                                     xT_sb[:, kt, csl],
                                     start=(kt == 0), stop=(kt == NKT - 1))
                if second:
                    eng, cast_eng = KP_ENG[c]
                    rope_to(kT_sb[:, csl], k_ps[0:HD, :], c, eng,
                            cast_eng or nc.vector)
                    del proj_ps[("k", c)]

            def vp_unit(c, tt):
                # V directly as [tok, hd]: x-tile stationary, wv moving.
                tsl = slice((c * 4 + tt) * 128, (c * 4 + tt + 1) * 128)
                v_ps = pj_pool.tile([128, CH], F32, tag="pj", name="v_ps")
                for kt in range(NKT):
                    nc.tensor.matmul(v_ps[:, 0:HD], xT_sb[:, kt, tsl],
                                     wv_sb[:, kt, :],
                                     start=(kt == 0), stop=(kt == NKT - 1))
                nc.vector.tensor_copy(v_sb[:, c * 4 + tt, :], v_ps[:, 0:HD])

            avn_tiles = {}    # (c, h) -> avn tile

            def wo_half(c, u, second, tail=False):
                tt, dc = u // 2, u % 2
                if not second:
                    proj_ps[("o", c, u)] = pjh_pool.tile([128, CH], F32,
                                                         tag="pjh", name="o_ps")
                o_ps = proj_ps[("o", c, u)]
                for h in ((0, 1) if not second else (2, 3)):
                    nc.tensor.matmul(
                        o_ps[:], avn_tiles[(c, h)][:, tt * 128:(tt + 1) * 128],
                        wo_sb[:, h, dc * CH:(dc + 1) * CH],
                        start=(h == 0), stop=(h == NHL - 1))
                if not second:
                    return
                del proj_ps[("o", c, u)]
                dst = out[c * CH + tt * 128: c * CH + (tt + 1) * 128,
                          dc * CH:(dc + 1) * CH]
                # DMA can't source PSUM (nor can gpsimd): stage to SBUF bf16
                # (halves the out DMA; host sums partials in fp32). Tail
                # units alternate staging between the then-idle ACT engine
                # and DVE, and split DMA issues across both queues (the
                # ~600ns/DMA issue cost is serial per queue).
                o_sb = osb_pool.tile([128, CH], BF16, tag="osb", name="o_sb")
                if tail and u % 2 == 1:
                    nc.scalar.copy(o_sb[:], o_ps[:])
                else:
                    nc.vector.tensor_copy(o_sb[:], o_ps[:])
                eng = nc.scalar if (tail and u % 2 == 1) else nc.sync
                eng.dma_start(dst, o_sb[:])

            # ---- filler scheduling ----
            # Split units hold their accumulating PSUM tile (pjh_pool, one
            # bank) across the gap between their halves; only ONE split may
            # be in flight, so any new split (or an out-of-band completion)
            # first flushes the previous one's second half.
            emitted = set()
            inflight = [None]   # pending second-half unit, or None

            def completion_of(u):
                k = u[0]
                if k in ("qp1", "qp2"):
                    return ("qp2", u[1], u[2])
                if k in ("kp1", "kp2"):
                    return ("kp2", u[1])
                if k in ("wo1", "wo2", "wot"):
                    return ("wo2", u[1], u[2])
                return None

            def emit_unit(u):
                if u in emitted:
                    return
                kind = u[0]
                u2 = completion_of(u)
                if u2 is not None:
                    fl = inflight[0]
                    if fl is not None and fl != u2:
                        inflight[0] = None
                        emit_unit(fl)
                emitted.add(u)
                if kind == "qp1":
                    qp_half(u[1], u[2], False)
                    inflight[0] = ("qp2", u[1], u[2])
                elif kind == "qp2":
                    emit_unit(("qp1", u[1], u[2]))
                    inflight[0] = None
                    qp_half(u[1], u[2], True)
                elif kind == "kp1":
                    kp_half(u[1], False)
                    inflight[0] = ("kp2", u[1])
                elif kind == "kp2":
                    emit_unit(("kp1", u[1]))
                    inflight[0] = None
                    kp_half(u[1], True)
                elif kind == "vp":
                    vp_unit(u[1], u[2])
                elif kind == "wo1":
                    wo_half(u[1], u[2], False)
                    inflight[0] = ("wo2", u[1], u[2])
                elif kind == "wo2":
                    emit_unit(("wo1", u[1], u[2]))
                    inflight[0] = None
                    wo_half(u[1], u[2], True)
                elif kind == "wot":
                    emit_unit(("wo1", u[1], u[2]))
                    inflight[0] = None
                    wo_half(u[1], u[2], True, tail=True)

            def qp(c, h):
                return [("qp1", c, h), ("qp2", c, h)]

            def kp(c):
                return [("kp1", c), ("kp2", c)]

            def wo(c, u):
                return [("wo1", c, u), ("wo2", c, u)]

            # estimated PE-us per sub-unit, for the paced emitter
            UCOST = {"qp1": 0.86, "qp2": 0.86, "kp1": 0.86, "kp2": 0.86,
                     "vp": 0.70, "wo1": 0.46, "wo2": 0.46, "wot": 0.46}

            # per-chunk filler lists, ordered by need-time. kp/vp of k-chunk
            # g must land before chunk tasks reach group g (groups advance
            # ~4us/task within head 0); qp(c,h) ~4 slots before head h's
            # tasks; wo(c') rides under the NEXT chunk's ACT stretch,
            # weighted late so its avn inputs are complete.
            fillers = {}
            fillers[2] = (kp(1) + qp(2, 1) + kp(2)
                          + [("vp", 0, tt) for tt in range(4)]
                          + qp(2, 2) + kp(3)
                          + [("vp", 1, tt) for tt in range(4)]
                          + qp(2, 3)
                          + [("vp", 2, tt) for tt in range(4)]
                          + qp(3, 0) + qp(3, 1))
            fillers[3] = (qp(3, 2) + qp(3, 3)
                          + [("vp", 3, tt) for tt in range(4)]
                          + qp(1, 0) + qp(1, 1)
                          + wo(2, 0) + wo(2, 1) + wo(2, 2) + wo(2, 3)
                          + qp(1, 2) + qp(1, 3)
                          + wo(2, 4) + wo(2, 5) + wo(2, 6) + wo(2, 7))
            fillers[1] = (qp(0, 0) + qp(0, 1)
                          + wo(3, 0) + wo(3, 1) + wo(3, 2)
                          + qp(0, 2) + qp(0, 3)
                          + wo(3, 3) + wo(3, 4) + wo(3, 5) + wo(3, 6)
                          + wo(3, 7))
            fillers[0] = (wo(1, 0) + wo(1, 1) + wo(1, 2) + wo(1, 3)
                          + wo(1, 4) + wo(1, 5) + wo(1, 6) + wo(1, 7))

            # ---- attention task machinery ----
            pend = []       # lagged AV work queue: (c, h, g, p4_tile)
            AV_LAG = 2      # tasks between exp(i) and its AV consumption
            head_acc = {}   # (c, h) -> (av_ps, d_ps), allocated at g == 0

            def emit_av(c, h, g, p4):
                """AV + quad-compress + ones-matmul for task (c,h,g); the
                consuming accumulators live across the head's groups."""
                diag = g == c
                for tt in range(4):
                    emit_unit(("vp", g, tt))
                if g == 0:
                    av_ps = acc_pool.tile([HD, CH], F32, tag="av", name="av_ps")
                    d_ps = d_pool.tile([128, CH], F32, tag="d", name="d_ps")
                    head_acc[(c, h)] = (av_ps, d_ps)
                av_ps, d_ps = head_acc[(c, h)]
                for j in range(4):
                    kb = 4 * g + j
                    if diag:
                        lo, po = 128 * j, DOFF[j]
                        w = CH - lo
                        nc.tensor.matmul(av_ps[:, lo:CH], v_sb[:, kb, :],
                                         p4[:, po:po + w],
                                         start=(kb == 0),
                                         stop=(g == c and j == 3))
                    else:
                        nc.tensor.matmul(av_ps[:], v_sb[:, kb, :],
                                         p4[:, j * CH:(j + 1) * CH],
                                         start=(kb == 0), stop=False)
                # quad-compress for the denominator: 3 adds -> 1 ones-MM
                ppq = pp_pool.tile([128, CH], BF16, tag="ppq", name="ppq")
                if diag:
                    nc.vector.tensor_copy(ppq[:], p4[:, 0:CH])
                    for j in range(1, 4):
                        lo = 128 * j
                        nc.vector.tensor_add(
                            ppq[:, lo:CH], ppq[:, lo:CH],
                            p4[:, DOFF[j]:DOFF[j] + (CH - lo)])
                else:
                    ppa = pp_pool.tile([128, CH], BF16, tag="ppa", name="ppa")
                    nc.vector.tensor_add(ppa[:], p4[:, 0:CH], p4[:, CH:2 * CH])
                    ppb = pp_pool.tile([128, CH], BF16, tag="ppb", name="ppb")
                    nc.vector.tensor_add(ppb[:], p4[:, 2 * CH:3 * CH],
                                         p4[:, 3 * CH:4 * CH])
                    nc.vector.tensor_add(ppq[:], ppa[:], ppb[:])
                nc.tensor.matmul(d_ps[:], ones_sb[:], ppq[:],
                                 start=(g == 0), stop=(g == c))
                if g == c:
                    # head (c,h) complete: normalize
                    dinv = norm_pool.tile([128, CH], F32, tag="dinv",
                                          name="dinv")
                    nc.vector.reciprocal_approx_fast(dinv[:], d_ps[:])
                    avn = avn_pool.tile([HD, CH], BF16, tag="avn", name="avn")
                    nc.vector.tensor_mul(avn[:], av_ps[:], dinv[:])
                    avn_tiles[(c, h)] = avn

            def emit_scores(c, h, g):
                """scores -> tanh -> exp(-> tri mask) for task (c,h,g)."""
                diag = g == c
                emit_unit(("kp2", g))
                qt = qt_tiles[c]
                s_t = s_pool.tile([128, DW], F32, tag="s", name="s_t")
                t4 = t4_pool.tile([128, DW], F32, tag="t4", name="t4")
                p4 = p_pool.tile([128, DW], BF16, tag="p4", name="p4")
                if diag:
                    for j in range(4):
                        kb = 4 * g + j
                        lo, po = 128 * j, DOFF[j]
                        w = CH - lo
                        nc.tensor.matmul(
                            s_t[:, po:po + w],
                            kT_sb[:, kb * 128:(kb + 1) * 128],
                            qt[:, h, lo:CH], start=True, stop=True)
                    nw = NWD
                else:
                    for j in range(4):
                        kb = 4 * g + j
                        nc.tensor.matmul(
                            s_t[:, j * CH:(j + 1) * CH],
                            kT_sb[:, kb * 128:(kb + 1) * 128],
                            qt[:, h, :], start=True, stop=True)
                    nw = DW
                nc.scalar.activation(t4[:, 0:nw], s_t[:, 0:nw],
                                     mybir.ActivationFunctionType.Tanh)
                nc.scalar.activation(p4[:, 0:nw], t4[:, 0:nw],
                                     mybir.ActivationFunctionType.Exp,
                                     scale=SOFTCAP)
                if diag:
                    # mask the four partially-visible 128-col triangles
                    for j in range(4):
                        po = DOFF[j]
                        nc.vector.tensor_mul(p4[:, po:po + 128],
                                             p4[:, po:po + 128], tri_sb[:])
                return p4

            # ---- main schedule ----
            # prologue: just enough for the first task (2,0,0). K first: x(c0)
            # + wk land ~3us before x(c2), so the PE clock ramp starts earlier.
            emit_unit(("kp2", 0))
            emit_unit(("qp2", 2, 0))

            for ci, c in enumerate(CSEQ):
                if ci >= 1:
                    # previous chunk's fillers must all have landed
                    for u in fillers[CSEQ[ci - 1]]:
                        emit_unit(u)
                fq = [u for u in fillers[c] if u not in emitted]
                credit = 0.0
                lag = AV_LAG if ci < len(CSEQ) - 1 else 1
                for h in range(NHL):
                    emit_unit(("qp2", c, h))   # safety; normally filler-paced
                    for g in range(c + 1):
                        p4 = emit_scores(c, h, g)
                        pend.append((c, h, g, p4))
                        while len(pend) > lag:
                            emit_av(*pend.pop(0))
                        # paced filler emission: each task slot's ACT pair
                        # covers its own scores + a lagged AV; the leftover
                        # budget (us of PE) goes to fillers, so no filler
                        # bundle ever delays the next task's score matmuls
                        # by more than ~one sub-unit.
                        credit += 2.0 if g != c else 1.2
                        while fq and UCOST[fq[0][0]] <= credit:
                            u = fq.pop(0)
                            if u in emitted:
                                continue
                            credit -= UCOST[u[0]]
                            emit_unit(u)
            while pend:
                emit_av(*pend.pop(0))
            for u in fillers[CSEQ[-1]]:
                emit_unit(u)
            for u in range(8):
                emit_unit(("wot", 0, u))

    nc.compile()
    return nc


_CACHED_NC = None


def _get_nc():
    global _CACHED_NC
    if _CACHED_NC is None:
        _CACHED_NC = _build_nc()
    return _CACHED_NC


def _host_inputs(x, Wq, Wk, Wv, Wo, qk_gain, cos, sin):
    """Build the 8 per-core input maps (bf16 matmul operands)."""
    x = np.asarray(x, np.float32)
    Wq = np.asarray(Wq, np.float32)
    Wk = np.asarray(Wk, np.float32)
    Wv = np.asarray(Wv, np.float32)
    Wo = np.asarray(Wo, np.float32)
    qk_gain = np.asarray(qk_gain, np.float32)
    cos = np.asarray(cos, np.float32)
    sin = np.asarray(sin, np.float32)

    scale = 1.0 / (np.sqrt(HD) * SOFTCAP)
    # Fold per-head gain and softcap scale into Wq rows.
    Wq_s = Wq * (qk_gain[:, None].repeat(HD, 1).reshape(NH * HD, 1) * scale)

    wkT = np.ascontiguousarray(Wk.T.astype(NPBF16))
    wvT = np.ascontiguousarray(Wv.T.astype(NPBF16))
    cosT = cos.T  # [64, T]
    sinT = sin.T
    cc = np.ascontiguousarray(np.concatenate([cosT, cosT], 0).astype(NPBF16))
    # m2 = swap(q) * ssw with swap done via copies: ssw = [-sin; sin]
    ssw = np.ascontiguousarray(np.concatenate([-sinT, sinT], 0).astype(NPBF16))

    # triangular mask for the diagonal 128-blocks: tri[kk, qq] = qq >= kk
    kk = np.arange(128)
    tri = (kk[None, :] >= kk[:, None]).astype(NPBF16)
    onesv = np.ones((128, 128), NPBF16)

    xTs = [np.ascontiguousarray(x[b].T.astype(NPBF16)) for b in range(B)]
    in_maps = []
    for core in range(8):
        b, hh = divmod(core, 2)
        h0 = hh * NHL
        wqT = np.ascontiguousarray(
            Wq_s[h0 * HD:(h0 + NHL) * HD, :].T.astype(NPBF16))
        woT = np.ascontiguousarray(
            Wo[:, h0 * HD:(h0 + NHL) * HD].T.astype(NPBF16))
        in_maps.append({
            "xT": xTs[b], "wqT": wqT, "wkT": wkT, "wvT": wvT, "woT": woT,
            "cc": cc, "ssw": ssw, "tri": tri, "onesv": onesv,
        })
    return in_maps


def kernel(x, Wq, Wk, Wv, Wo, qk_gain, cos, sin, _trace=False):
    in_maps = _host_inputs(x, Wq, Wk, Wv, Wo, qk_gain, cos, sin)
    nc = _get_nc()
    res = run_bass_kernel_spmd(nc, in_maps, core_ids=list(range(8)),
                               trace=_trace)
    out = np.empty((B, T, D), np.float32)
    for b in range(B):
        out[b] = (res.results[2 * b]["out"].astype(np.float32)
                  + res.results[2 * b + 1]["out"].astype(np.float32))
    if _trace:
        kernel.last_exec_time_ns = res.exec_time_ns
        kernel.last_results = res
    return out
